# revision 1
# baseline (speedup 1.0000x reference)
"""Trainium2 Bass kernel for nn_Decoder_19172734009903.

Decomposition (validated in numpy to 2e-4 rel err with bf16 weights):
  - embedding lookup via indirect DMA gather (bf16 table)
  - char-CNN = one packed projection matmul [tok,256]@[256,768] on X.T,
    then shifted adds along free (token) axis + segment max + relu(bias)
  - segment means via matmul with a block-mean matrix A (tokens contracted)
  - gx = dec_in @ W_ih.T precomputed for all segments (big matmul)
  - LSTM scan: per step W_hh.T streamed as 64 bf16 [128,128] stationary
    tiles (FWL), gates in [gate-part, batch-free] layout for cheap cell math
  - fc + log_softmax batched after the scan from stored h history

Sharding: G batch-groups x S segment-chunks across 8 cores. Chunks s>0
start W segments early from zero state (LSTM contraction makes truncation
error ~ 0.5^W); chunk 0 runs W dummy segments then resets to the true h0
via a per-core data mask (same SPMD program on every core).
"""

import numpy as np
import ml_dtypes

import concourse.bass as bass
import concourse.mybir as mybir
import concourse.tile as tile
from concourse import bass_utils

BF16 = ml_dtypes.bfloat16

# sharding config (env override is for experimentation only)
import os as _os

G_BATCH = int(_os.environ.get("KCFG_G", "8"))   # batch groups
S_CHUNK = int(_os.environ.get("KCFG_S", "1"))   # segment chunks (G*S == 8)
WARMUP = int(_os.environ.get("KCFG_W", "0"))    # warmup segs (NS must be %16)

B, SEQ, D, H2, F, V, T, L = 64, 1024, 256, 512, 128, 50000, 64, 8
NSEG = SEQ // L                  # 128
NCORES = 8

b = B // G_BATCH                 # seqs per core
nseg_own = NSEG // S_CHUNK       # owned segments per core
NS = nseg_own + WARMUP           # processed segments per core
NT = NS * L                      # tokens per seq per core
R = b * NS                       # feature rows (seq-major: q*NS + s)
R_OUT = b * nseg_own             # output rows per core
NCH = NT // 128                  # 128-token chunks per seq
FP32 = mybir.dt.float32
DBF = mybir.dt.bfloat16

# gate m-tile -> column block (units of b) so that sigma runs on [i f o]
# contiguously and tanh on [g]:  i:0-3 -> 0-3, f:4-7 -> 4-7, o:12-15 -> 8-11,
# g:8-11 -> 12-15
def _gcol(m):
    if m < 8:
        return m
    if m >= 12:
        return m - 4
    return m + 4


def _split_multi_waits(nc):
    """This container's walrus codegen accepts at most ONE on_wait per
    instruction ("Too many sync wait commands"); hoist extra waits onto
    preceding same-engine NoOps."""
    k = 0
    for fn in nc.m.functions:
        for blk in fn.blocks:
            new = []
            for inst in blk.instructions:
                si = inst.sync_info
                if si is not None and si.on_wait and len(si.on_wait) > 1:
                    waits = list(si.on_wait)
                    for w in waits[:-1]:
                        k += 1
                        nop = mybir.InstNoOp(name=f"I-waitsplit-{k}", ins=[], outs=[])
                        nop.engine = inst.engine
                        nop.sync_info = mybir.SyncInfo(on_wait=[w], on_update=[])
                        new.append(nop)
                    inst.sync_info = mybir.SyncInfo(
                        on_wait=[waits[-1]], on_update=list(si.on_update)
                    )
                new.append(inst)
            blk.instructions = new
    return k


def build_program():
    nc = bass.Bass("TRN2", target_bir_lowering=False, debug=False)

    def din(name, shape, dt):
        return nc.dram_tensor(name, shape, dt, kind="ExternalInput").ap()

    wid_d = din("wid", [128, b * NCH], mybir.dt.int32)   # [p, chunk] token ids
    ident_d = din("ident", [128, 128], FP32)              # identity for PE transpose
    emb_d = din("emb", [V, D], FP32)
    enc_d = din("enc", [b, NT, H2], DBF)
    wcat_d = din("wcat", [128, 2, 6 * F], DBF)           # packed conv proj
    a8_d = din("a8", [128, 16], DBF)
    a8f_d = din("a8f", [128, 16], FP32)                     # block-mean matrix
    bias_d = din("bias", [128, 3], FP32)                 # conv biases
    wih_d = din("wih", [128, 9, 2048], DBF)              # W_ih.T tiles
    whh_d = din("whh", [128, 4, 2048], DBF)              # W_hh.T tiles
    wfc_d = din("wfc", [128, 4, T], DBF)                 # W_fc.T tiles
    hinit_d = din("hinit", [128, 4 * b], FP32)           # state at t=0
    h0m_d = din("h0m", [128, 4 * b], FP32)               # (1-m)*h0 for t=W reset
    m_d = din("m", [128, 1], FP32)                       # warm-keep mask
    out_d = nc.dram_tensor("out", [R_OUT, T], FP32, kind="ExternalOutput").ap()
    import os
    dbg = os.environ.get("KERNEL_DEBUG_TAPS", "0") == "1"
    if dbg:
        dbg_din = nc.dram_tensor("dbg_din", [128, 9, R], DBF, kind="ExternalOutput").ap()
        dbg_gx = nc.dram_tensor("dbg_gx", [128, 16, R], DBF, kind="ExternalOutput").ap()
        dbg_hh = nc.dram_tensor("dbg_hh", [128, 4, R_OUT], DBF, kind="ExternalOutput").ap()
        dbg_proj = nc.dram_tensor("dbg_proj", [128, 6, NT], DBF, kind="ExternalOutput").ap()
        dbg_xt = nc.dram_tensor("dbg_xt", [128, 2, NT], DBF, kind="ExternalOutput").ap()

    with tile.TileContext(nc) as tc:
        with (
            tc.tile_pool(name="consts", bufs=1) as consts,
            tc.tile_pool(name="work", bufs=2) as work,
            tc.tile_pool(name="gath", bufs=3) as gath,
            tc.tile_pool(name="cell", bufs=2) as cellp,
            tc.tile_pool(name="psA", bufs=2, space="PSUM") as psA,
            tc.tile_pool(name="psB", bufs=2, space="PSUM") as psB,
            tc.tile_pool(name="psG", bufs=2, space="PSUM") as psG,
            tc.tile_pool(name="psT", bufs=2, space="PSUM") as psT,
        ):
            # ---- load weights/constants ----
            wid_sb = consts.tile([128, b * NCH], mybir.dt.int32)
            nc.sync.dma_start(wid_sb, wid_d)
            wcat_sb = consts.tile([128, 2, 6 * F], DBF)
            nc.sync.dma_start(wcat_sb, wcat_d)
            a8_sb = consts.tile([128, 16], DBF)
            nc.sync.dma_start(a8_sb, a8_d)
            a8f_sb = consts.tile([128, 16], FP32)
            nc.sync.dma_start(a8f_sb, a8f_d)
            bias_sb = consts.tile([128, 3], FP32)
            nc.sync.dma_start(bias_sb, bias_d)
            wih_sb = consts.tile([128, 9, 2048], DBF)
            nc.sync.dma_start(wih_sb, wih_d)
            whh_sb = consts.tile([128, 4, 2048], DBF)
            nc.sync.dma_start(whh_sb, whh_d)
            wfc_sb = consts.tile([128, 4, T], DBF)
            nc.sync.dma_start(wfc_sb, wfc_d)
            m_sb = consts.tile([128, 1], FP32)
            nc.sync.dma_start(m_sb, m_d)
            ident_sb = consts.tile([128, 128], FP32)
            nc.sync.dma_start(ident_sb, ident_d)
            h0m_sb = consts.tile([128, 4 * b], FP32)
            nc.sync.dma_start(h0m_sb, h0m_d)

            dinT = consts.tile([128, 9, R], DBF)     # [c_w(2) c_h(4) c_x(3)]
            gx = consts.tile([128, 16, NS, b], DBF)  # precomputed W_ih @ din, t-major
            h_hist = consts.tile([128, 4, R_OUT], DBF)

            skip_gather = os.environ.get("KSKIP_GATHER", "0") == "1"
            skip_xt = os.environ.get("KSKIP_XT", "0") == "1"
            # ---- per-sequence feature extraction ----
            for q in range(b):
                x_rows = work.tile([128, NCH, D], FP32, tag="x_rows")
                if skip_gather:
                    nc.vector.memset(x_rows, 0.0)
                else:
                    for c in range(NCH):
                        nc.gpsimd.indirect_dma_start(
                            out=x_rows[:, c, :],
                            out_offset=None,
                            in_=emb_d,
                            in_offset=bass.IndirectOffsetOnAxis(
                                ap=wid_sb[:, q * NCH + c : q * NCH + c + 1], axis=0
                            ),
                        )
                # X.T via DMA transpose: [tok,256] -> [256(2p), tok]
                xt = work.tile([128, 2, NT], DBF, tag="xt")
                if skip_xt:
                    nc.vector.memset(xt, 0.0)
                else:
                    for c in range(NCH):
                        for d1 in range(2):
                            ps_t = psT.tile([128, 128], FP32, tag="ps_t")
                            nc.tensor.transpose(
                                ps_t,
                                x_rows[:, c, d1 * 128 : (d1 + 1) * 128],
                                ident_sb,
                            )
                            nc.vector.tensor_copy(
                                out=xt[:, d1, c * 128 : (c + 1) * 128], in_=ps_t
                            )
                # encoder rows for this seq
                e_sb = work.tile([128, NCH, H2], DBF, tag="e_sb")
                nc.sync.dma_start(
                    e_sb, enc_d[q].rearrange("(c p) f -> p c f", p=128)
                )
                # c_w.T and c_h.T via block-mean matmuls (tokens contracted)
                for d1 in range(2):
                    ps = psB.tile([128, NCH * 16], FP32, tag="ps_mean")
                    for c in range(NCH):
                        nc.tensor.matmul(
                            out=ps[:, c * 16 : (c + 1) * 16],
                            lhsT=x_rows[:, c, d1 * 128 : (d1 + 1) * 128],
                            rhs=a8f_sb,
                            start=True, stop=True,
                        )
                    nc.scalar.copy(
                        out=dinT[:, d1, q * NS : q * NS + NS], in_=ps[:, :NS]
                    )
                for d4 in range(4):
                    ps = psB.tile([128, NCH * 16], FP32, tag="ps_mean")
                    for c in range(NCH):
                        nc.tensor.matmul(
                            out=ps[:, c * 16 : (c + 1) * 16],
                            lhsT=e_sb[:, c, d4 * 128 : (d4 + 1) * 128],
                            rhs=a8_sb,
                            start=True, stop=True,
                        )
                    nc.scalar.copy(
                        out=dinT[:, 2 + d4, q * NS : q * NS + NS], in_=ps[:, :NS]
                    )
                # conv projections proj.T [6F, tok]
                projT = work.tile([128, 6, NT], DBF, tag="projT")
                for f6 in range(6):
                    for n0 in range(0, NT, 512):
                        nn = min(512, NT - n0)
                        ps = psA.tile([128, 512], FP32, tag="ps_proj")
                        for d1 in range(2):
                            nc.tensor.matmul(
                                out=ps[:, :nn],
                                lhsT=wcat_sb[:, d1, f6 * F : (f6 + 1) * F],
                                rhs=xt[:, d1, n0 : n0 + nn],
                                start=(d1 == 0), stop=(d1 == 1),
                            )
                        nc.scalar.copy(out=projT[:, f6, n0 : n0 + nn], in_=ps[:, :nn])
                # shifted adds + segment max + relu(bias)
                y2 = work.tile([128, NT], DBF, tag="y2")
                nc.vector.tensor_tensor(
                    out=y2[:, : NT - 1], in0=projT[:, 1, : NT - 1],
                    in1=projT[:, 2, 1:NT], op=mybir.AluOpType.add,
                )
                y3 = work.tile([128, NT], DBF, tag="y3")
                nc.vector.tensor_tensor(
                    out=y3[:, : NT - 2], in0=projT[:, 3, : NT - 2],
                    in1=projT[:, 4, 1 : NT - 1], op=mybir.AluOpType.add,
                )
                nc.vector.tensor_tensor(
                    out=y3[:, : NT - 2], in0=y3[:, : NT - 2],
                    in1=projT[:, 5, 2:NT], op=mybir.AluOpType.add,
                )
                mx = work.tile([128, 3, NS], FP32, tag="mx")
                nc.vector.tensor_reduce(
                    out=mx[:, 0], in_=projT[:, 0].rearrange("p (s l) -> p s l", l=L),
                    axis=mybir.AxisListType.X, op=mybir.AluOpType.max,
                )
                nc.vector.tensor_reduce(
                    out=mx[:, 1],
                    in_=y2.rearrange("p (s l) -> p s l", l=L)[:, :, : L - 1],
                    axis=mybir.AxisListType.X, op=mybir.AluOpType.max,
                )
                nc.vector.tensor_reduce(
                    out=mx[:, 2],
                    in_=y3.rearrange("p (s l) -> p s l", l=L)[:, :, : L - 2],
                    axis=mybir.AxisListType.X, op=mybir.AluOpType.max,
                )
                for j in range(3):
                    nc.scalar.activation(
                        out=dinT[:, 6 + j, q * NS : q * NS + NS], in_=mx[:, j],
                        func=mybir.ActivationFunctionType.Relu,
                        bias=bias_sb[:, j : j + 1],
                    )
                if dbg and q == 0:
                    nc.sync.dma_start(dbg_proj, projT)
                    nc.sync.dma_start(dbg_xt, xt)

            # ---- gx = W_ih @ dec_in for all rows ----
            CS = max(1, 512 // NS) * NS  # N-chunk aligned to whole sequences
            QS = CS // NS
            for mt in range(16):
                for n0 in range(0, R, CS):
                    nn = min(CS, R - n0)
                    ps = psA.tile([128, 512], FP32, tag="ps_proj")
                    for k in range(9):
                        nc.tensor.matmul(
                            out=ps[:, :nn],
                            lhsT=wih_sb[:, k, mt * 128 : (mt + 1) * 128],
                            rhs=dinT[:, k, n0 : n0 + nn],
                            start=(k == 0), stop=(k == 8),
                        )
                    nc.vector.tensor_copy(
                        out=gx[:, _gcol(mt), :, n0 // NS : n0 // NS + nn // NS],
                        in_=ps[:, :nn].rearrange("p (q t) -> p t q", t=NS),
                    )

            if os.environ.get("KERNEL_PHASES", "all") == "features":
                _split_multi_waits(nc)
                return nc

            # ---- LSTM scan ----
            h_f = consts.tile([128, 4 * b], FP32)
            nc.sync.dma_start(h_f, hinit_d)
            h_bf = consts.tile([128, 4 * b], DBF)
            nc.vector.tensor_copy(out=h_bf, in_=h_f)
            c_f = consts.tile([128, 4 * b], FP32)
            nc.vector.memset(c_f, 0.0)

            for t in range(NS):
                if WARMUP and t == WARMUP:
                    # state reset: h = h*m + (1-m)*h0 ; c = c*m
                    nc.vector.tensor_scalar_mul(h_f, h_f, m_sb[:, 0:1])
                    nc.vector.tensor_add(out=h_f, in0=h_f, in1=h0m_sb)
                    nc.vector.tensor_scalar_mul(c_f, c_f, m_sb[:, 0:1])
                    nc.vector.tensor_copy(out=h_bf, in_=h_f)
                ps_g = psG.tile([128, 16 * b], FP32, tag="ps_g")
                for mt in range(16):
                    col = _gcol(mt) * b
                    for k in range(4):
                        nc.tensor.matmul(
                            out=ps_g[:, col : col + b],
                            lhsT=whh_sb[:, k, mt * 128 : (mt + 1) * 128],
                            rhs=h_bf[:, k * b : (k + 1) * b],
                            start=(k == 0), stop=(k == 3),
                        )
                g_sb = cellp.tile([128, 16 * b], FP32, tag="g_sb")
                gxt = gx[:, :, t, :]  # [128, 16, b]
                nc.vector.tensor_tensor(
                    out=g_sb.rearrange("p (m q) -> p m q", q=b),
                    in0=ps_g.rearrange("p (m q) -> p m q", q=b),
                    in1=gxt, op=mybir.AluOpType.add,
                )
                sig = cellp.tile([128, 12 * b], FP32, tag="sig")
                nc.scalar.activation(
                    out=sig, in_=g_sb[:, : 12 * b],
                    func=mybir.ActivationFunctionType.Sigmoid,
                )
                tg = cellp.tile([128, 4 * b], FP32, tag="tg")
                nc.scalar.activation(
                    out=tg, in_=g_sb[:, 12 * b :],
                    func=mybir.ActivationFunctionType.Tanh,
                )
                t1 = cellp.tile([128, 4 * b], FP32, tag="t1")
                nc.vector.tensor_tensor(
                    out=t1, in0=sig[:, : 4 * b], in1=tg, op=mybir.AluOpType.mult
                )
                nc.vector.tensor_tensor(
                    out=c_f, in0=sig[:, 4 * b : 8 * b], in1=c_f,
                    op=mybir.AluOpType.mult,
                )
                nc.vector.tensor_add(out=c_f, in0=c_f, in1=t1)
                tc_ = cellp.tile([128, 4 * b], FP32, tag="tc")
                nc.scalar.activation(
                    out=tc_, in_=c_f, func=mybir.ActivationFunctionType.Tanh
                )
                nc.vector.tensor_tensor(
                    out=h_f, in0=sig[:, 8 * b : 12 * b], in1=tc_,
                    op=mybir.AluOpType.mult,
                )
                nc.vector.tensor_copy(out=h_bf, in_=h_f)
                if t >= WARMUP:
                    tt = t - WARMUP
                    hh = h_hist.rearrange("p k (q t) -> p k t q", t=nseg_own)
                    nc.vector.tensor_copy(
                        out=hh[:, :, tt, :],
                        in_=h_bf.rearrange("p (k q) -> p k q", q=b),
                    )

            # ---- fc + log_softmax over stored h ----
            for r0 in range(0, R_OUT, 128):
                rn = min(128, R_OUT - r0)
                ps_l = psB.tile([128, T], FP32, tag="ps_mean")
                for k in range(4):
                    nc.tensor.matmul(
                        out=ps_l[:rn],
                        lhsT=h_hist[:, k, r0 : r0 + rn],
                        rhs=wfc_sb[:, k, :],
                        start=(k == 0), stop=(k == 3),
                    )
                nmx = cellp.tile([128, 1], FP32, tag="nmx")
                nc.vector.tensor_reduce(
                    out=nmx[:rn], in_=ps_l[:rn], axis=mybir.AxisListType.X,
                    op=mybir.AluOpType.max, negate=True,
                )
                ex = cellp.tile([128, T], FP32, tag="ex")
                se = cellp.tile([128, 1], FP32, tag="se")
                nc.scalar.activation(
                    out=ex[:rn], in_=ps_l[:rn],
                    func=mybir.ActivationFunctionType.Exp,
                    bias=nmx[:rn], accum_out=se[:rn],
                )
                lse = cellp.tile([128, 1], FP32, tag="lse")
                nc.scalar.activation(
                    out=lse[:rn], in_=se[:rn],
                    func=mybir.ActivationFunctionType.Ln,
                )
                off = cellp.tile([128, 1], FP32, tag="off")
                nc.vector.tensor_tensor(
                    out=off[:rn], in0=nmx[:rn], in1=lse[:rn],
                    op=mybir.AluOpType.subtract,
                )
                o_sb = cellp.tile([128, T], FP32, tag="o_sb")
                nc.vector.tensor_scalar_add(o_sb[:rn], ps_l[:rn], off[:rn, 0:1])
                nc.sync.dma_start(out_d[r0 : r0 + rn], o_sb[:rn])

            if dbg:
                nc.sync.dma_start(dbg_din, dinT)
                nc.sync.dma_start(dbg_gx, gx)
                nc.sync.dma_start(dbg_hh, h_hist)

    _split_multi_waits(nc)
    return nc


_cached = None


def _get_program():
    global _cached
    if _cached is None:
        _cached = build_program()
    return _cached


def _prep_core_inputs(inputs, core):
    """Host-side slicing / packing of inputs for one core (layout only)."""
    g_idx, s_idx = core // S_CHUNK, core % S_CHUNK
    q0 = g_idx * b
    wi = np.asarray(inputs["word_ids"], np.int32)
    enc = np.asarray(inputs["encoder_outputs"], np.float32)

    seg0 = s_idx * nseg_own
    if s_idx == 0:
        # W dummy warmup segments (replicate first own segment); state is
        # reset to the true h0 at t=WARMUP via the mask.
        tsl = np.concatenate(
            [np.tile(np.arange(L), WARMUP), np.arange(nseg_own * L)]
        )
        tok_idx = tsl + seg0 * L
    else:
        tok_idx = np.arange((seg0 - WARMUP) * L, (seg0 + nseg_own) * L)

    wid = wi[q0 : q0 + b][:, tok_idx]                       # [b, NT]
    wid_p = np.ascontiguousarray(
        wid.reshape(b * NCH, 128).T.reshape(128, b * NCH)
    ).astype(np.int32)
    encs = np.ascontiguousarray(enc[q0 : q0 + b][:, tok_idx]).astype(BF16)

    h0 = np.concatenate(
        [enc[q0 : q0 + b, -1, : H2 // 2], enc[q0 : q0 + b, 0, H2 // 2 :]], axis=1
    ).astype(np.float32)                                    # [b, 512]
    h0T = np.ascontiguousarray(h0.T).reshape(4, 128, b).transpose(1, 0, 2)
    h0T = np.ascontiguousarray(h0T.reshape(128, 4 * b), dtype=np.float32)
    if s_idx == 0:
        hinit = h0T if WARMUP == 0 else np.zeros_like(h0T)
        h0m = h0T
        m = np.zeros((128, 1), np.float32)
    else:
        hinit = np.zeros_like(h0T)
        h0m = np.zeros_like(h0T)
        m = np.ones((128, 1), np.float32)
    return {"wid": wid_p, "enc": encs, "hinit": hinit, "h0m": h0m, "m": m}


def kernel(**inputs):
    nc = _get_program()

    emb = np.ascontiguousarray(np.asarray(inputs["embed"], np.float32))
    w1 = np.asarray(inputs["wconv1"], np.float32)
    w2 = np.asarray(inputs["wconv2"], np.float32)
    w3 = np.asarray(inputs["wconv3"], np.float32)
    wcat = np.concatenate(
        [w1[:, 0].T, w2[:, 0].T, w2[:, 1].T, w3[:, 0].T, w3[:, 1].T, w3[:, 2].T],
        axis=1,
    )  # [256, 768]
    wcat_p = np.ascontiguousarray(
        wcat.reshape(2, 128, 6 * F).transpose(1, 0, 2)
    ).astype(BF16)
    a8 = np.zeros((128, 16), np.float32)
    for s in range(16):
        a8[s * 8 : (s + 1) * 8, s] = 1.0 / L
    a8 = a8.astype(BF16)
    biases = np.stack(
        [np.asarray(inputs["bconv1"]), np.asarray(inputs["bconv2"]),
         np.asarray(inputs["bconv3"])], axis=1
    ).astype(np.float32)  # [128, 3]
    wih = np.ascontiguousarray(
        np.asarray(inputs["W_ih"], np.float32).T.reshape(9, 128, 2048)
    ).astype(BF16)
    whh = np.ascontiguousarray(
        np.asarray(inputs["W_hh"], np.float32).T.reshape(4, 128, 2048)
    ).astype(BF16)
    wfc = np.ascontiguousarray(
        np.asarray(inputs["W_fc"], np.float32).T.reshape(4, 128, T)
    ).astype(BF16)
    shared = {
        "emb": emb,
        "ident": np.eye(128, dtype=np.float32),
        "wcat": wcat_p,
        "a8": a8,
        "a8f": a8.astype(np.float32),
        "bias": biases,
        "wih": wih.transpose(1, 0, 2).copy(),
        "whh": whh.transpose(1, 0, 2).copy(),
        "wfc": wfc.transpose(1, 0, 2).copy(),
    }
    in_maps = []
    for core in range(NCORES):
        im = dict(shared)
        im.update(_prep_core_inputs(inputs, core))
        in_maps.append(im)

    import os

    trace = os.environ.get("BASS_TRACE_RUN", "0") == "1"
    res = bass_utils.run_bass_kernel_spmd(
        nc, in_maps, core_ids=list(range(NCORES)), trace=trace
    )
    global LAST_RESULTS
    LAST_RESULTS = res
    out = np.zeros((B * NSEG, T), np.float32)
    for core in range(NCORES):
        o = res.results[core]["out"]  # [R_OUT, T] rows (q, t) seq-major
        g_idx, s_idx = core // S_CHUNK, core % S_CHUNK
        o = o.reshape(b, nseg_own, T)
        for q in range(b):
            r0 = (g_idx * b + q) * NSEG + s_idx * nseg_own
            out[r0 : r0 + nseg_own] = o[q]
    return out

